# revision 3
# baseline (speedup 1.0000x reference)
"""Trainium2 Bass kernel for the dense branch-MLP problem (fp8 DoubleRow).

Computes: out[b,o] = sum_n relu((s[b,:] - v[n,:]) @ W[n].T + bias[n])[o]
with B=1024, N=64, D=512, OUT=2048 in fp32.

Sharding: expert-style across the N=64 branch axis -> 8 branches per core.
Each core computes a full [B, OUT] partial sum over its 8 branches; the
host sums the 8 partials (the unshard step).

Reformulation: the vertex offset folds into a per-(n,o) constant
  c[n,o] = bias[n,o] - v[n] @ W[n].T          (host, float64)
and the per-branch device work is
  acc[o,b] += relu((s @ (16 W[n]).T)[b,o] + 16 c[n,o])
with s and 16*W quantized host-side to fp8-e4m3 and all matmuls in
DoubleRow perf mode (2 fp8 MACs/cell/cycle, contract 256/matmul). The x16
scaling keeps W out of the e4m3 subnormal range; the host divides the
gathered partials by 16. Validated numerically vs the f64 reference:
rel-absmax err ~1.31e-2 (gate 2e-2).

Structure (the non-obvious bits):
  - The bias 16c is injected INTO PSUM by a third DoubleRow matmul per
    (branch, b-half): stationary = [c_hi; c_lo] on partition 0 (hi/lo fp8
    split, so the bias lands at ~e4m3^2 precision = negligible), moving =
    ones. This makes the relu pass bias-free, so ONE activation op can
    span a 2048-wide PSUM pair-tile covering TWO branches (per-op
    overhead halves, and no per-branch [128,1]-bias constraint).
  - Loop is o-tile-OUTER, branch-INNER: all 8 branches of one o-tile
    complete together, so output DMAs trickle out once per block instead
    of bunching into a serial 16-DMA tail, and the gpsimd engine has
    add-work from the first block.
  - Branch-sum is a pair tree: each relu op already covers a branch pair
    [n | n+1] in one fp16 tile; half-adds fold the pairs, then two more
    levels produce the block output. Adds are fp16 (DVE 2x_1p eligible)
    split DVE/gpsimd by a weighted counter; the relu pass splits ACT/DVE.
  - All weights SBUF-resident (64KB/partition, streamed once at start);
    s is resident (4KB); there is no per-branch DVE offset work at all.

Cost-model engine budget per core: PE 768 matmuls ~82us, ACT ~85us,
DVE ~85us, gpsimd ~85us -> ~92us with startup+tail, vs 235.6us fp32r
baseline.
"""

import numpy as np

import concourse.bacc as bacc
import concourse.mybir as mybir
import concourse.tile as tile
from concourse.bass_utils import run_bass_kernel_spmd

B, N, D, OUT = 1024, 64, 512, 2048
N_CORES = 8
NL = N // N_CORES  # branches per core (8)
NP = NL // 2       # branch pairs per core (4)
OT = OUT // 128    # o tiles (16)
C2 = 2             # contract super-chunks of 256 (DoubleRow)
J = 2              # fp8 pair dim per super-chunk
BT = 2             # b halves of 512 (one PSUM bank each)
CJO = C2 * J * OUT # flat wt free size per branch (8192)

F32 = mybir.dt.float32
F16 = mybir.dt.float16
F8 = mybir.dt.float8e4
BF16 = mybir.dt.bfloat16
RELU = mybir.ActivationFunctionType.Relu
DR = mybir.MatmulPerfMode.DoubleRow
ADD = mybir.AluOpType.add
MAX = mybir.AluOpType.max

# Engine-split weights (fraction routed to the first-listed engine),
# balanced against the cost model: relu ACT/DVE, L1 adds gpsimd/DVE.
R_ACT_W = 0.705
A_POOL_W = 0.625

_cache = {}


def build(repeat: int = 1):
    """Build + compile the per-core Bass program. Cached per `repeat`."""
    if repeat in _cache:
        return _cache[repeat]

    nc = bacc.Bacc(
        "TRN2",
        target_bir_lowering=False,
        debug=False,
        num_devices=N_CORES,
    )

    wt_d = nc.dram_tensor("wt", [NL, 128, CJO], F8, kind="ExternalInput").ap()
    st_d = nc.dram_tensor("st", [128, C2 * J * B], F8, kind="ExternalInput").ap()
    # c8[p, n, j, o]: partition 0 holds j=0: e4m3(16c), j=1: e4m3(16c - hi);
    # partitions 1..127 are zero (the bias matmul contracts over all 128).
    c8_d = nc.dram_tensor("c8", [128, NL * 2 * 128 * OT], F8, kind="ExternalInput").ap()
    out_d = nc.dram_tensor("out", [OUT, B], F16, kind="ExternalOutput").ap()

    with tile.TileContext(nc) as tc:
        with (
            tc.tile_pool(name="const", bufs=1) as const_pool,
            tc.tile_pool(name="outp", bufs=3) as out_pool,
            tc.tile_pool(name="tmp", bufs=8) as tmp_pool,
            tc.tile_pool(name="psum", bufs=2, space="PSUM") as psum_pool,
        ):
            # Startup DMA order: the first pair-tile needs st(c2=0), the
            # first wt chunk, c8, then st(c2=1).
            st = const_pool.tile([128, C2 * J * B], F8, name="st")
            nc.sync.dma_start(st[:, 0 : J * B], st_d[:, 0 : J * B])
            wt = const_pool.tile([128, NL * CJO], F8, name="wt")
            wt5 = wt[:].rearrange("p (n c j o) -> p n c j o", n=NL, c=C2, j=J)
            wd4 = wt_d[0].rearrange("p (c j o) -> p c j o", c=C2, j=J)
            nc.sync.dma_start(wt5[:, 0, :, :, 0:1024], wd4[:, :, :, 0:1024])
            c8 = const_pool.tile([128, NL * 2 * 128 * OT], F8, name="c8")
            nc.sync.dma_start(c8[:], c8_d[:])
            nc.sync.dma_start(st[:, J * B : 2 * J * B], st_d[:, J * B : 2 * J * B])
            nc.sync.dma_start(wt5[:, 0, :, :, 1024:2048], wd4[:, :, :, 1024:2048])
            for n in range(1, NL):
                nc.sync.dma_start(wt[:, n * CJO : (n + 1) * CJO], wt_d[n])

            st4 = st[:].rearrange("p (c j b) -> p c j b", c=C2, j=J)
            c8v = c8[:].rearrange("p (n j t o) -> p n j t o", n=NL, j=2, t=OT)

            # ones moving operand for the bias matmuls (fp8 1.0 is exact).
            ones = const_pool.tile([128, J * 512], F8, name="ones")
            nc.gpsimd.memset(ones[:], 1.0)
            ones3 = ones[:].rearrange("p (j b) -> p j b", j=J)

            # PE warmup burst during the startup DMA window (HAM clock gate),
            # and an early Relu to hide the ACT table load.
            scr = const_pool.tile([128, 128], BF16, name="scr")
            nc.vector.memset(scr[:], 0.0)
            pre = const_pool.tile([128, 8], F16, name="pre")
            nc.scalar.activation(pre[:], scr[:, 0:8], RELU, bias=0.0, scale=1.0)
            wps = psum_pool.tile([128, 2048], F32, name="wps", tag="ps")
            for _ in range(56):
                nc.tensor.matmul(
                    wps[0:64, 0:64], scr[:, 0:64], scr[:, 64:128], start=True, stop=True
                )

            def body(iv=None):
                r_bal = 0.0
                a_bal = 0.0
                for ot in range(OT):
                    last_block = ot == OT - 1
                    ups = []  # per-pair folded [128, B] fp16 slices
                    for np_ in range(NP):
                        # pair-tile: [n2, bt, 512] quarters, each a PSUM bank
                        ps = psum_pool.tile([128, 2048], F32, name="ps", tag="ps")
                        for n2 in range(2):
                            n = np_ * 2 + n2
                            for c2 in range(C2):
                                lhsT = wt5[:, n, c2, :, ot * 128 : (ot + 1) * 128]
                                for bt in range(BT):
                                    q = n2 * 1024 + bt * 512
                                    nc.tensor.matmul(
                                        ps[:, q : q + 512],
                                        lhsT,
                                        st4[:, c2, :, bt * 512 : (bt + 1) * 512],
                                        start=(c2 == 0),
                                        stop=False,
                                        perf_mode=DR,
                                    )
                            cb = c8v[:, n, :, ot, :]
                            for bt in range(BT):
                                q = n2 * 1024 + bt * 512
                                nc.tensor.matmul(
                                    ps[:, q : q + 512],
                                    cb,
                                    ones3[:],
                                    start=False,
                                    stop=True,
                                    perf_mode=DR,
                                )
                        # relu pass: one op over both branches of the pair
                        tmp = tmp_pool.tile([128, 2048], F16, name="tmp", tag="tmp")
                        r_bal += R_ACT_W
                        if r_bal >= 1.0:
                            r_bal -= 1.0
                            nc.scalar.activation(
                                tmp[:], ps[:], RELU, bias=0.0, scale=1.0
                            )
                        else:
                            nc.vector.tensor_scalar(tmp[:], ps[:], 0.0, None, MAX)
                        # L1: fold the branch pair (halves of the same tile)
                        u = tmp[:, 0:B]
                        a_bal += A_POOL_W
                        if a_bal >= 1.0 and not last_block:
                            a_bal -= 1.0
                            nc.gpsimd.tensor_add(u, u, tmp[:, B : 2 * B])
                        else:
                            nc.vector.tensor_add(u, u, tmp[:, B : 2 * B])
                        ups.append(u)
                    # L2 + L3 (DVE): ((p0+p1) + (p2+p3)) -> out tile -> DMA
                    nc.vector.tensor_add(ups[0], ups[0], ups[1])
                    nc.vector.tensor_add(ups[2], ups[2], ups[3])
                    ob = out_pool.tile([128, B], F16, name="ob", tag="ob")
                    nc.vector.tensor_add(ob[:], ups[0], ups[2])
                    nc.sync.dma_start(out_d[ot * 128 : (ot + 1) * 128, :], ob[:])

            if repeat == 1:
                body()
            else:
                with tc.For_i(0, repeat, 1):
                    body()

    nc.compile()
    _cache[repeat] = nc
    return nc


def prep_inputs(semantic_vec, vertices, W, b):
    """Host-side quantization + layout transforms -> per-core input maps."""
    s = np.asarray(semantic_vec, dtype=np.float32)
    v = np.asarray(vertices, dtype=np.float32)
    W = np.asarray(W, dtype=np.float32)
    bb = np.asarray(b, dtype=np.float32)
    f8 = mybir.dt.np(F8)

    # st[p, c2, j, bb] = s[bb, c2*256 + j*128 + p]
    st = np.ascontiguousarray(
        s.reshape(B, C2, J, 128).transpose(3, 1, 2, 0).reshape(128, C2 * J * B)
    ).astype(f8)
    # wt[n][p, c2, j, o] = 16 * W[n, o, c2*256 + j*128 + p]
    wt = np.ascontiguousarray(
        (W * np.float32(16.0))
        .reshape(N, OUT, C2, J, 128)
        .transpose(0, 4, 2, 3, 1)
        .reshape(N, 128, CJO)
    ).astype(f8)
    # c8[p=0, n, j, ot, o]: j=0 -> e4m3(16c), j=1 -> e4m3(16c - hi); rows 1..127 zero
    c = np.empty((N, OUT), dtype=np.float64)
    v64 = v.astype(np.float64)
    for n in range(N):
        c[n] = bb[n].astype(np.float64) - W[n].astype(np.float64) @ v64[n]
    c16 = (16.0 * c).astype(np.float32)
    c_hi = c16.astype(f8)
    c_lo = (c16 - c_hi.astype(np.float32)).astype(f8)
    c8 = np.zeros((N_CORES, 128, NL, 2, OUT), dtype=f8)
    c8[:, 0, :, 0, :] = c_hi.reshape(N_CORES, NL, OUT)
    c8[:, 0, :, 1, :] = c_lo.reshape(N_CORES, NL, OUT)
    c8 = np.ascontiguousarray(c8.reshape(N_CORES, 128, NL * 2 * OUT))

    in_maps = []
    for core in range(N_CORES):
        in_maps.append(
            {
                "wt": wt[core * NL : (core + 1) * NL],
                "st": st,
                "c8": c8[core],
            }
        )
    return in_maps


def kernel(semantic_vec, vertices, W, b):
    nc = build(repeat=1)
    in_maps = prep_inputs(semantic_vec, vertices, W, b)
    res = run_bass_kernel_spmd(nc, in_maps, core_ids=list(range(N_CORES)))
    total = np.zeros((OUT, B), dtype=np.float32)
    for core in range(N_CORES):
        total += np.asarray(res.results[core]["out"]).astype(np.float32)
    total *= 1.0 / 16.0
    return np.ascontiguousarray(total.T)


# revision 29
# speedup vs baseline: 1.4425x; 1.4425x over previous
"""Trainium2 Bass kernel for the dense branch-MLP problem (fp8 DoubleRow).

Computes: out[b,o] = sum_n relu((s[b,:] - v[n,:]) @ W[n].T + bias[n])[o]
with B=1024, N=64, D=512, OUT=2048 in fp32.

Sharding: expert-style across the N=64 branch axis -> 8 branches per core.
Each core computes a full [B, OUT] partial sum over its 8 branches; the
host sums the 8 partials (the unshard step).

Reformulation: the vertex offset folds into a per-(n,o) constant
  c[n,o] = bias[n,o] - v[n] @ W[n].T          (host, float64)
and the per-branch device work is
  acc[o,b] += relu((s @ (16 W[n]).T)[b,o] + 16 c[n,o])
with s and 16*W quantized host-side to fp8-e4m3 and the matmuls run in
DoubleRow perf mode (2 fp8 MACs/cell/cycle, contract 256 -> 2 matmuls per
[128o x 512b] PSUM bank instead of 4). The x16 scaling keeps W out of the
e4m3 subnormal range; the host divides the gathered partials by 16.
Validated numerically vs the f64 reference: rel-absmax err ~1.30e-2
(gate 2e-2); fp16 on-chip accumulation adds <1e-4.

Structure (lessons baked in from the cost-model traces):
  - Per-branch [128,1024] PSUM tiles, 4 in rotation: a 2-deep rotation of
    2048-wide pair-tiles made the pipeline latency-bound (fill had to
    wait a ~2us relu two tiles back -> 1.64us/tile > the 1.34 budget).
  - relu+bias is one op per branch: ACT activation(Relu, bias=16c) or
    DVE tensor_scalar(add 16c, max 0), split ~68/32 by a weighted
    counter. Branch-sums fold as a pair tree: u = relu_n + relu_n+1
    (in place over the tmp tiles), then acc[ot] += u; adds are fp16
    (DVE tensor_tensor hits the 2x_1p mode) split ~62/38 DVE/gpsimd.
  - Two-phase schedule: pairs 0/1 run pair-major (only the first weight
    chunks needed -> fast start while the weight stream lands); pairs
    2/3 run ot-major so each ot's final add + output DMA stagger across
    the entire second half instead of bunching into a serial tail.
  - The cost model serializes all DMA through one engine pool at
    ~360B/ns, so transfers are contiguous-only (strided chunks pay 2x),
    ordered by first use: c16, st(c2=0), wt0[ot0-7], st(c2=1),
    wt1[ot0-7], then the rest. wt host layout is ot-major per branch so
    a contiguous prefix covers leading o-tiles.
  - PE warmup burst + an early dummy Relu (hides the 1.3us ACT table
    load) run during the startup DMA window.

Cost-model engine budget per core: ACT ~90us, DVE ~90us, gpsimd ~90us,
PE ~58us (512 DoubleRow matmuls + warmup), DMA pipe ~38us.
"""

import numpy as np

import concourse.bacc as bacc
import concourse.mybir as mybir
import concourse.tile as tile
from concourse.bass_utils import run_bass_kernel_spmd

B, N, D, OUT = 1024, 64, 512, 2048
N_CORES = 8
NL = N // N_CORES  # branches per core (8)
NP = NL // 2       # branch pairs per core (4)
OT = OUT // 128    # o tiles (16)
C2 = 2             # contract super-chunks of 256 (DoubleRow)
J = 2              # fp8 pair dim per super-chunk
BT = 2             # b halves of 512 (one PSUM bank each)
CJO = C2 * J * OUT # flat wt free size per branch (8192)

F32 = mybir.dt.float32
F16 = mybir.dt.float16
F8 = mybir.dt.float8e4
BF16 = mybir.dt.bfloat16
RELU = mybir.ActivationFunctionType.Relu
DR = mybir.MatmulPerfMode.DoubleRow
ADD = mybir.AluOpType.add
MAX = mybir.AluOpType.max

# Engine-split weights, balanced against the cost model.
# relu: ACT vs DVE. adds: DVE tensor_add vs offloaded; offloaded adds go to
# a gpsimd-issued CCE accumulate-DMA (SBUF->SBUF add in the DMA path,
# ~1.0us of Pool engine + ~0.7us of DMA pipe) with a small gpsimd
# tensor_add remainder.
R_ACT_W = 0.620
A_OFFLOAD_W = 0.660
A_CCE_W = 0.892

_cache = {}


def build(repeat: int = 1):
    """Build + compile the per-core Bass program. Cached per `repeat`."""
    if repeat in _cache:
        return _cache[repeat]

    nc = bacc.Bacc(
        "TRN2",
        target_bir_lowering=False,
        debug=False,
        num_devices=N_CORES,
    )

    # wt host layout: [n, p, t(16), c2, j, o128] — ot-major so a contiguous
    # prefix of a branch delivers complete leading o-tiles.
    wt_d = nc.dram_tensor("wt", [NL, 128, CJO], F8, kind="ExternalInput").ap()
    st_d = nc.dram_tensor("st", [128, C2 * J * B], F8, kind="ExternalInput").ap()
    c16_d = nc.dram_tensor("c16", [128, NL * OT], F32, kind="ExternalInput").ap()
    out_d = nc.dram_tensor("out", [OUT, B], F16, kind="ExternalOutput").ap()

    HB = CJO // 2  # half-branch chunk (o-tiles 0..7 or 8..15)

    with tile.TileContext(nc) as tc:
        with (
            tc.tile_pool(name="const", bufs=1) as const_pool,
            tc.tile_pool(name="acc", bufs=1) as acc_pool,
            tc.tile_pool(name="tmp", bufs=16) as tmp_pool,
            tc.tile_pool(name="psum", bufs=4, space="PSUM") as psum_pool,
        ):
            # The DMA engine pool is serialized in the cost model: order the
            # queue by first use.
            c16 = const_pool.tile([128, NL * OT], F32, name="c16")
            nc.sync.dma_start(c16[:], c16_d[:])
            st = const_pool.tile([128, C2 * J * B], F8, name="st")
            nc.sync.dma_start(st[:], st_d[:])
            wt = const_pool.tile([128, NL * CJO], F8, name="wt")
            nc.sync.dma_start(wt[:, 0:HB], wt_d[0][:, 0:HB])
            nc.sync.dma_start(wt[:, CJO : CJO + HB], wt_d[1][:, 0:HB])
            for n in range(2):
                nc.sync.dma_start(
                    wt[:, n * CJO + HB : (n + 1) * CJO], wt_d[n][:, HB:CJO]
                )
            for n in range(2, NL):
                nc.sync.dma_start(wt[:, n * CJO : n * CJO + HB], wt_d[n][:, 0:HB])
                nc.sync.dma_start(
                    wt[:, n * CJO + HB : (n + 1) * CJO], wt_d[n][:, HB:CJO]
                )

            st4 = st[:].rearrange("p (c j b) -> p c j b", c=C2, j=J)
            wt6 = wt[:].rearrange(
                "p (n t c j o) -> p n t c j o", n=NL, t=OT, c=C2, j=J
            )

            acc = [
                acc_pool.tile([128, B], F16, name=f"acc{ot}", tag=f"acc{ot}")
                for ot in range(OT)
            ]

            # PE warmup burst during the startup DMA window (HAM clock gate),
            # and an early Relu to hide the ACT table load.
            scr = const_pool.tile([128, 128], BF16, name="scr")
            nc.vector.memset(scr[:], 0.0)
            pre = const_pool.tile([128, 8], F16, name="pre")
            nc.scalar.activation(pre[:], scr[:, 0:8], RELU, bias=0.0, scale=1.0)
            wps = psum_pool.tile([128, B], F32, name="wps", tag="ps")
            for _ in range(72):
                nc.tensor.matmul(
                    wps[0:64, 0:64], scr[:, 0:64], scr[:, 64:128], start=True, stop=True
                )

            def body(iv=None):
                bal = {"r": 0.0, "a": 0.0, "c": 0.0}

                def add(dst, in0, in1, eligible=True, inplace=True):
                    bal["a"] += A_OFFLOAD_W
                    if bal["a"] >= 1.0 and eligible:
                        bal["a"] -= 1.0
                        bal["c"] += A_CCE_W
                        if bal["c"] >= 1.0 and inplace:
                            bal["c"] -= 1.0
                            # CCE accumulate: dst += in1 in the DMA path
                            nc.gpsimd.dma_start(dst, in1, accum_op=ADD)
                        else:
                            nc.gpsimd.tensor_add(dst, in0, in1)
                    else:
                        nc.vector.tensor_add(dst, in0, in1)

                def branch_tile(n, ot, dst=None, force_r=None):
                    """matmuls + relu for one branch -> dst (or a fresh tmp)."""
                    ps = psum_pool.tile([128, B], F32, name="ps", tag="ps")
                    for c2 in range(C2):
                        lhsT = wt6[:, n, ot, c2, :, :]
                        for bt in range(BT):
                            nc.tensor.matmul(
                                ps[:, bt * 512 : (bt + 1) * 512],
                                lhsT,
                                st4[:, c2, :, bt * 512 : (bt + 1) * 512],
                                start=(c2 == 0),
                                stop=(c2 == C2 - 1),
                                perf_mode=DR,
                            )
                    b_ap = c16[:, n * OT + ot : n * OT + ot + 1]
                    if dst is None:
                        dst = tmp_pool.tile([128, B], F16, name="tmp", tag="tmp")[:]
                    if force_r is None:
                        bal["r"] += R_ACT_W
                        use_act = bal["r"] >= 1.0
                        if use_act:
                            bal["r"] -= 1.0
                    else:
                        use_act = force_r == "act"
                    if use_act:
                        nc.scalar.activation(dst, ps[:], RELU, bias=b_ap, scale=1.0)
                    else:
                        nc.vector.tensor_scalar(dst, ps[:], b_ap, 0.0, ADD, MAX)
                    return dst

                # adds are emitted 2 blocks late: each engine's FIFO then
                # sees relu ops (which gate the shared PSUM ring) promptly,
                # with adds as back-pressure-free filler behind them.
                pending = []

                def flush(keep=0):
                    while len(pending) > keep:
                        for f in pending.pop(0):
                            f()

                def pair_block(np_, ot):
                    # the final ots' add-chains gate the kernel tail: keep
                    # them off the CCE round-trip and alternate their relus
                    # ACT-first so the two engines drain the tail in parallel
                    fast_tail = np_ == NP - 1 and ot >= OT - 2
                    fr = ("act", "dve") if fast_tail else (None, None)
                    if np_ == 0:
                        # branch 0's relu writes acc directly; no fold tile
                        branch_tile(0, ot, dst=acc[ot][:])
                        t1 = branch_tile(1, ot)
                        laters = [lambda: add(acc[ot][:], acc[ot][:], t1)]
                    else:
                        t0 = branch_tile(np_ * 2, ot, force_r=fr[0])
                        t1 = branch_tile(np_ * 2 + 1, ot, force_r=fr[1])
                        laters = [
                            lambda: add(t0, t0, t1, eligible=not fast_tail),
                            lambda: add(
                                acc[ot][:], acc[ot][:], t0, eligible=not fast_tail
                            ),
                        ]
                    if np_ == NP - 1:
                        laters.append(
                            lambda: nc.sync.dma_start(
                                out_d[ot * 128 : (ot + 1) * 128, :], acc[ot][:]
                            )
                        )
                    pending.append(laters)
                    flush(keep=2)

                # Phase A: pairs 0/1 pair-major (needs only early weight
                # chunks). Phase B: pairs 2/3 ot-major with pair 3 trailing
                # pair 2 by one ot, so each ot's acc-chain (2,ot)->(3,ot) has
                # a full block of slack and completions/DMAs stagger across
                # the whole second half.
                for np_ in (0, 1):
                    for ot in range(OT):
                        pair_block(np_, ot)
                pair_block(2, 0)
                for ot in range(1, OT):
                    pair_block(2, ot)
                    pair_block(3, ot - 1)
                pair_block(3, OT - 1)
                flush()

            if repeat == 1:
                body()
            else:
                with tc.For_i(0, repeat, 1):
                    body()

    nc.compile()
    _cache[repeat] = nc
    return nc


def prep_inputs(semantic_vec, vertices, W, b):
    """Host-side quantization + layout transforms -> per-core input maps."""
    s = np.asarray(semantic_vec, dtype=np.float32)
    v = np.asarray(vertices, dtype=np.float32)
    W = np.asarray(W, dtype=np.float32)
    bb = np.asarray(b, dtype=np.float32)
    f8 = mybir.dt.np(F8)

    # st[p, c2, j, bb] = s[bb, c2*256 + j*128 + p]
    st = np.ascontiguousarray(
        s.reshape(B, C2, J, 128).transpose(3, 1, 2, 0).reshape(128, C2 * J * B)
    ).astype(f8)
    # wt[n][p, ot, c2, j, o] = 16 * W[n, ot*128 + o, c2*256 + j*128 + p]
    wt = np.ascontiguousarray(
        (W * np.float32(16.0))
        .reshape(N, OT, 128, C2, J, 128)
        .transpose(0, 5, 1, 3, 4, 2)
        .reshape(N, 128, CJO)
    ).astype(f8)
    # c16[core][p, nl*OT + ot] = 16 * (b[n] - v[n] @ W[n].T)[ot*128 + p]
    c = np.empty((N, OUT), dtype=np.float64)
    v64 = v.astype(np.float64)
    for n in range(N):
        c[n] = bb[n].astype(np.float64) - W[n].astype(np.float64) @ v64[n]
    c16 = np.ascontiguousarray(
        (16.0 * c)
        .astype(np.float32)
        .reshape(N_CORES, NL, OT, 128)
        .transpose(0, 3, 1, 2)
        .reshape(N_CORES, 128, NL * OT)
    )

    in_maps = []
    for core in range(N_CORES):
        in_maps.append(
            {
                "wt": wt[core * NL : (core + 1) * NL],
                "st": st,
                "c16": c16[core],
            }
        )
    return in_maps


def kernel(semantic_vec, vertices, W, b):
    nc = build(repeat=1)
    in_maps = prep_inputs(semantic_vec, vertices, W, b)
    res = run_bass_kernel_spmd(nc, in_maps, core_ids=list(range(N_CORES)))
    total = np.zeros((OUT, B), dtype=np.float32)
    for core in range(N_CORES):
        total += np.asarray(res.results[core]["out"]).astype(np.float32)
    total *= 1.0 / 16.0
    return np.ascontiguousarray(total.T)
